# revision 28
# baseline (speedup 1.0000x reference)
"""Trainium2 Bass kernel for nn_Attention_48601849922045.

GQA attention layer (B=2, S=2048, D=2048, H=32 q-heads, KV=8 kv-heads, HD=64)
with llama RoPE, causal softmax, and output projection.

Sharding: tensor-parallel over heads across 8 cores - each core owns one KV
group (1 kv head + its 4 q heads).  x is replicated; per core:

  1. QKV projection, weights-stationary, e-tile-outer with the 16 x-chunks
     of an n-block resident in SBUF (2 PSUM accumulators instead of 3).
  2. RoPE in [e, n] layout (pair-swap via strided SBUF->SBUF DMAs on the
     otherwise idle Scalar DMA ring, 3 DVE ops); in-place on the q tiles.
     v is PE-transposed to [t, hd] with a ones column for the softmax
     denominator.
  3. Attention per (b, head-PAIR): the two heads' K=64 scores matmuls are
     issued back-to-back at base partitions 0/64 so they run CONCURRENTLY
     in disjoint PE row groups (2x scores throughput vs per-head).  Exp on
     ACT covers both heads in one call, trimmed to the causal width.
     PV per head into separate PSUM banks, causally sliced; denominator
     from the ones column, inverted on DVE, broadcast by a K=1 matmul.
  4. FOUR bf16 AllToAlls (one per (batch, head-pair), 512KB each) reshard
     o from head-sharded to row-sharded, preceded by a tiny warmup
     collective at kernel start that absorbs the ncfw barrier+pipeline
     establishment cost (~60us on the first collective otherwise).
  5. Row-parallel output projection (256 rows per batch half), b0 passes
     interleaved into the batch-1 attention, b1 split into hp-waves so
     only the hp1 half waits on the final collective.

PSUM plan (8 banks): psmm 2 (qkv accumulators + v-transpose + outproj
accumulators - phases never overlap), pssp 2x[P,2,JW] = 4 (double-buffered
pair-scores), pso 2 (per-head o).  DMA rings: sync = x-chunks + staging +
outputs, gpsimd = weights + b0 x-chunks + collective waits + orT loads,
vector = consts + wo tiles + b1 x-chunks, scalar = rope swaps + k-dup.

Host side only shards/transposes inputs and concatenates the 8 output
row-shards.
"""

import os

import numpy as np
import ml_dtypes

import concourse.bass as bass
import concourse.bacc as bacc
import concourse.tile as tile
import concourse.mybir as mybir
from concourse.bass_utils import run_bass_kernel_spmd

P = 128
B, S, D = 2, 2048, 2048
H, KV, HD = 32, 8, 64
NCORES = 8
HL = H // NCORES          # 4 local q heads
BS = B * S                # 4096 rows
EQ, EK, EV = HL * HD, HD, HD
E3 = EQ + EK + EV         # 384 = 3 PE tiles of 128
ET = E3 // P              # 3 e-tiles (0,1: q head pairs, 2: k|v stacked)
CH = D // P               # 16 contraction chunks
STC = S // P              # 16 t-chunks per batch
NBW = 512                 # qkv n-block width
NBB = S // NBW            # 4 n-blocks per batch
JW = 512                  # attention n-block width
JB = S // JW
RSH = BS // NCORES // B   # 256 rows per (core, batch)
VAW = P                   # v-aug stride: ones at col 64, zero-padded
HP = 2                    # head pairs per core

F32 = mybir.dt.float32
BF16 = mybir.dt.bfloat16

_CACHE = {}


def _build_nc():
    nc = bacc.Bacc("TRN2", target_bir_lowering=False, debug=False,
                   num_devices=NCORES)

    xT = nc.dram_tensor("xT", [B * NBB, P, CH * NBW], BF16,
                        kind="ExternalInput").ap()
    wT = nc.dram_tensor("wT", [P, CH * E3], BF16, kind="ExternalInput").ap()
    woT = nc.dram_tensor("woT", [D // JW, P, CH * JW], BF16,
                         kind="ExternalInput").ap()
    cosT = nc.dram_tensor("cosT", [P, S], BF16, kind="ExternalInput").ap()
    sinPM = nc.dram_tensor("sinPM", [P, S], BF16, kind="ExternalInput").ap()
    mask2 = nc.dram_tensor("mask2", [P, 2 * P], BF16, kind="ExternalInput").ap()
    ident = nc.dram_tensor("ident", [P, P], BF16, kind="ExternalInput").ap()
    out = nc.dram_tensor("out", [B * RSH, D], F32, kind="ExternalOutput").ap()

    with tile.TileContext(nc) as tc:
        with (
            tc.tile_pool(name="const", bufs=1) as const,
            tc.tile_pool(name="dram", bufs=1, space="DRAM") as dram,
            # PSUM: psmm 2 banks (qkv acc / vtp / outproj acc), pssp 4,
            # pso 2 -> 8 banks exactly
            tc.tile_pool(name="psmm", bufs=2, space="PSUM") as psmm,
            tc.tile_pool(name="pssp", bufs=2, space="PSUM") as pssp,
            tc.tile_pool(name="pso", bufs=1, space="PSUM") as pso,
            tc.tile_pool(name="xg", bufs=2) as xgp,
            tc.tile_pool(name="drain", bufs=2) as drainp,
            tc.tile_pool(name="ptp", bufs=3) as ptp,
            tc.tile_pool(name="nrm", bufs=2) as nrm,
            tc.tile_pool(name="otp", bufs=2) as otp,
            tc.tile_pool(name="wosA", bufs=1) as wosA,
            tc.tile_pool(name="wosB", bufs=1) as wosB,
            tc.tile_pool(name="wosC", bufs=1) as wosC,
            tc.tile_pool(name="wosD", bufs=1) as wosD,
            tc.tile_pool(name="orp", bufs=1) as orp,
            tc.tile_pool(name="outs", bufs=2) as outsp,
        ):
            # ---- constants; warmup collective (absorbs the ncfw
            # barrier) AFTER wT so its trigger stall doesn't block it;
            # tiny exp loads the ACT table set ----
            wT_sb = const.tile([P, CH * E3], BF16)
            HW = CH * E3 // 2
            nc.gpsimd.dma_start(out=wT_sb[:, 0:HW], in_=wT[:, 0:HW])
            nc.sync.dma_start(out=wT_sb[:, HW:], in_=wT[:, HW:])
            warm_in = dram.tile([NCORES, 16], BF16)
            warm_out = dram.tile([NCORES, 16], BF16)

            def warm_collective():
                nc.gpsimd.collective_compute(
                    "AllToAll", mybir.AluOpType.bypass,
                    replica_groups=[list(range(NCORES))],
                    ins=[warm_in.opt()], outs=[warm_out.opt()])
            wexp = const.tile([1, 8], F32)
            nc.vector.memset(wexp, 0.0)
            wexp2 = const.tile([1, 8], F32)
            nc.scalar.activation(out=wexp2, in_=wexp,
                                 func=mybir.ActivationFunctionType.Exp)
            cos_sb = const.tile([P, S], BF16)
            nc.sync.dma_start(out=cos_sb, in_=cosT)
            sin_sb = const.tile([P, S], BF16)
            nc.sync.dma_start(out=sin_sb, in_=sinPM)
            mask2_sb = const.tile([P, 2, P], BF16)
            nc.scalar.dma_start(
                out=mask2_sb, in_=mask2.rearrange("p (two c) -> p two c", two=2))
            id_sb = const.tile([P, P], BF16)
            nc.scalar.dma_start(out=id_sb, in_=ident)
            ones_sb = const.tile([1, HD], BF16)
            nc.vector.memset(ones_sb, 1.0)
            wos = {}

            sw_sh = const.tile([P, S], BF16, name="sw_sh")
            st = {}
            for b in range(B):
                st[b] = {
                    # q head-pairs, roped IN PLACE in [e, n] layout
                    "qr": [const.tile([P, S], BF16, name=f"qr{b}{i}")
                           for i in range(2)],
                    "kv": const.tile([P, S], BF16, name=f"kv{b}"),
                    # k stored twice (partitions 0-63 and 64-127) so the
                    # pair's scores lhsT hit disjoint PE row groups
                    "kr": const.tile([P, S], BF16, name=f"kr{b}"),
                    "sw": sw_sh,
                    "vA": const.tile([P, STC * VAW], BF16, name=f"vA{b}"),
                }
                nc.vector.memset(st[b]["vA"], 0.0)
                ones_col = st[b]["vA"].rearrange(
                    "p (t w) -> p t w", w=VAW)[:, :, HD:HD + 1]
                nc.vector.memset(ones_col, 1.0)

            a2a_in = dram.tile([B, HP, NCORES, 2, HD, RSH], BF16)
            a2a_out = dram.tile([B, HP, NCORES, 2, HD, RSH], BF16)

            orT = {(b, hp): orp.tile([P, CH // 2 * RSH], BF16,
                                     name=f"orT{b}{hp}", tag=f"orT{b}{hp}")
                   for b in range(B) for hp in range(HP)}

            def a2a(b, hp):
                nc.gpsimd.collective_compute(
                    "AllToAll",
                    mybir.AluOpType.bypass,
                    replica_groups=[list(range(NCORES))],
                    ins=[a2a_in[b, hp].opt()],
                    outs=[a2a_out[b, hp].opt()],
                )
                for s in range(NCORES):
                    nc.gpsimd.dma_start(
                        out=orT[(b, hp)][:, s * RSH:(s + 1) * RSH],
                        in_=a2a_out[b, hp, s].rearrange(
                            "two hd r -> (two hd) r"))

            args = (xT, wT_sb, cos_sb, sin_sb, id_sb)
            apair = (mask2_sb, ones_sb, a2a_in, pssp, pso, ptp, nrm, otp)

            # ---- Phase A: qkv(b0) + attn(0, hp0) interleaved; all b0
            # x-chunks ride the gpsimd ring BEFORE any collective blocks
            # its queue ----
            dq = []
            for nb in range(NBB):
                _qkv_block(nc, 0, nb, *args, st[0], xgp, psmm, drainp)
                while dq:
                    dq.pop(0)()
                _attn_pair(nc, 0, 0, st[0], *apair, j_range=[nb], defer=dq)
            warm_collective()
            while dq:
                dq.pop(0)()
            a2a(0, 0)
            # ---- Phase B: qkv(b1) + attn(0, hp1) ----
            for nb in range(NBB):
                _qkv_block(nc, 1, nb, *args, st[1], xgp, psmm, drainp)
                while dq:
                    dq.pop(0)()
                _attn_pair(nc, 0, 1, st[0], *apair, j_range=[nb], defer=dq)
                if nb == 1:
                    # wo col-blocks 0/1 ride the scalar ring behind the
                    # b1 x-chunks; ready well before phase C needs them
                    wos[0] = _load_wos(nc, woT, wosA, 0, nc.scalar)
                    wos[1] = _load_wos(nc, woT, wosB, 1, nc.scalar)
            while dq:
                dq.pop(0)()
            a2a(0, 1)
            wos[2] = _load_wos(nc, woT, wosC, 2, nc.scalar)
            wos[3] = _load_wos(nc, woT, wosD, 3, nc.scalar)
            # ---- Phase C: attn(1, hp0) + b0 outproj db0/db1 ----
            _attn_pair(nc, 1, 0, st[1], *apair, j_range=[0, 1])
            _outproj_pass(nc, 0, 0, out, orT, wos[0], psmm, outsp,
                          nc.vector.tensor_copy)
            _attn_pair(nc, 1, 0, st[1], *apair, j_range=[2, 3])
            _outproj_pass(nc, 1, 0, out, orT, wos[1], psmm, outsp,
                          nc.vector.tensor_copy)
            a2a(1, 0)
            # ---- Phase D: attn(1, hp1) + b0 outproj db2/db3 ----
            _attn_pair(nc, 1, 1, st[1], *apair, j_range=[0])
            _outproj_pass(nc, 2, 0, out, orT, wos[2], psmm, outsp,
                          nc.vector.tensor_copy)
            _attn_pair(nc, 1, 1, st[1], *apair, j_range=[1, 2])
            _attn_pair(nc, 1, 1, st[1], *apair, j_range=[3])
            a2a(1, 1)
            # db3's b0 pass runs during the final AllToAll (only needs
            # a2a(0,1)); together with the four hp0 waves it covers the
            # collective's full latency
            _outproj_pass(nc, 3, 0, out, orT, wos[3], psmm, outsp,
                          nc.vector.tensor_copy)
            # ---- Tail: all four b1 hp0 waves run during the final
            # AllToAll (a2a(1,0) data only), then the hp1 waves.  The
            # four accumulator pairs use psmm, pso, and the two freed
            # pssp slots - exactly 8 PSUM banks. ----
            acc = {}
            acc[0] = [psmm.tile([P, JW], F32, name=f"opA{mt}", tag="ps")
                      for mt in range(2)]
            acc[1] = [pso.tile([P, JW], F32, name="opB0", tag="o0"),
                      pso.tile([P, JW], F32, name="opB1", tag="o1")]
            spt2 = pssp.tile([P, 2, JW], F32, name="opC", tag="sp")
            acc[2] = [spt2[:, 0, :], spt2[:, 1, :]]
            spt3 = pssp.tile([P, 2, JW], F32, name="opD", tag="sp")
            acc[3] = [spt3[:, 0, :], spt3[:, 1, :]]
            for db in range(4):
                _outproj_wave(nc, db, 1, 0, out, orT, wos[db], acc[db],
                              outsp)
            for db in range(4):
                _outproj_wave(nc, db, 1, 1, out, orT, wos[db], acc[db],
                              outsp, drain=nc.scalar.copy)

    nc.compile()
    return nc


def _qkv_block(nc, b, nb, xT, wT_sb, cos_sb, sin_sb, id_sb, stb, xgp, psmm,
               drainp):
    """Weights-stationary projection for one 512-column n-block, e-tile
    outer with the 16 x-chunks resident, followed by rope / k-dup /
    v-transpose so attention on this block can start immediately."""
    nbg = b * NBB + nb
    n0 = nb * NBW
    TPB = NBW // P
    HC = CH // 2 * NBW
    xg = xgp.tile([P, CH * NBW], BF16)
    if b == 0:
        QX = CH // 4 * NBW
        if nb == 0:
            nc.scalar.dma_start(out=xg[:, 0:QX], in_=xT[nbg][:, 0:QX])
            nc.gpsimd.dma_start(out=xg[:, QX:HC], in_=xT[nbg][:, QX:HC])
        else:
            nc.gpsimd.dma_start(out=xg[:, 0:HC], in_=xT[nbg][:, 0:HC])
        nc.scalar.dma_start(out=xg[:, HC:], in_=xT[nbg][:, HC:])
    else:
        nc.scalar.dma_start(out=xg[:, 0:HC], in_=xT[nbg][:, 0:HC])
        nc.sync.dma_start(out=xg[:, HC:], in_=xT[nbg][:, HC:])
    cp = nc.scalar.copy if b == 0 else nc.vector.tensor_copy
    # e order (0, 2, 1): the k rope gates this block's scores, while the
    # second q pair isn't read until the NEXT phase - emit k|v before it
    for e in (0, 2, 1):
        ps = psmm.tile([P, NBW], F32, name=f"ps{e}", tag="ps")
        for c in range(CH):
            nc.tensor.matmul(
                ps[:, :],
                lhsT=wT_sb[:, c * E3 + e * P:c * E3 + (e + 1) * P],
                rhs=xg[:, c * NBW:(c + 1) * NBW],
                start=(c == 0), stop=(c == CH - 1))
        if e < 2:
            cp(out=stb["qr"][e][:, n0:n0 + NBW], in_=ps)
            _rope_t(nc, drainp, stb["qr"][e], stb["qr"][e], stb["sw"],
                    cos_sb, sin_sb, P, n0, ps=ps)
        else:
            cp(out=stb["kv"][:, n0:n0 + NBW], in_=ps)
            _rope_t(nc, drainp, stb["kv"], stb["kr"], stb["sw"],
                    cos_sb, sin_sb, HD, n0, ps=ps)
            nc.sync.dma_start(out=stb["kr"][HD:P, n0:n0 + NBW],
                                in_=stb["kr"][0:HD, n0:n0 + NBW])
            # v: PE transpose to natural [t, hd] + ones column
            vtp = psmm.tile([P, TPB * HD], BF16, name="vtp", tag="ps",
                            padded_shape=[P, 2 * TPB * HD])
            for tl in range(TPB):
                t = nb * TPB + tl
                nc.tensor.transpose(vtp[:, tl * HD:(tl + 1) * HD],
                                    stb["kv"][HD:P, t * P:(t + 1) * P],
                                    id_sb[HD:P, HD:P])
            vAv = stb["vA"].rearrange("p (t w) -> p t w", w=VAW)[:, :, 0:HD]
            nc.vector.tensor_copy(
                out=vAv[:, nb * TPB:(nb + 1) * TPB, :],
                in_=vtp.rearrange("p (t w) -> p t w", w=HD))


def _rope_t(nc, drainp, src, dst, sw, cos_sb, sin_sb, rows, n0, ps=None):
    """dst[0:rows, n0:n0+NBW] = rope(src[...]) in [e, n] layout (dst may
    alias src).  The cos-product reads the PSUM accumulator directly so it
    runs in parallel with the ACT drain; the pair-swap (strided
    SBUF->SBUF DMA on the sync ring) only gates the sin-product."""
    n1 = n0 + NBW
    t1 = drainp.tile([P, NBW], BF16, name="t1", tag="t1", bufs=2)
    t2 = drainp.tile([P, NBW], BF16, name="t2", tag="t2", bufs=2)
    nc.vector.tensor_mul(t1[0:rows], ps[0:rows, :] if ps is not None
                         else src[0:rows, n0:n1], cos_sb[0:rows, n0:n1])
    nc.sync.dma_start(out=sw[0:rows:2, n0:n1], in_=src[1:rows:2, n0:n1])
    nc.sync.dma_start(out=sw[1:rows:2, n0:n1], in_=src[0:rows:2, n0:n1])
    nc.vector.tensor_mul(t2[0:rows], sw[0:rows, n0:n1],
                         sin_sb[0:rows, n0:n1])
    nc.vector.tensor_add(dst[0:rows, n0:n1], t1[0:rows], t2[0:rows])


def _attn_pair(nc, b, hp, stb, mask2_sb, ones_sb, a2a_in, pssp, pso, ptp,
               nrm, otp, j_range=None, defer=None):
    """Causal attention for one (batch, head-pair): heads 2hp (partitions
    0-63) and 2hp+1 (64-127).  Scores of the two heads run concurrently
    in disjoint PE row groups; exp covers both heads, causally trimmed.

    When ``defer`` is a list, the per-j normalization/staging closure is
    appended to it instead of being emitted inline - the caller flushes
    it after the next qkv block's ropes so the DVE queue prioritizes
    rope over the (latency-tolerant) norm tail."""
    qrt = stb["qr"][hp]
    kr = stb["kr"]
    vA = stb["vA"]
    for j in (range(JB) if j_range is None else j_range):
        n0 = j * JW
        ni = (n0 + JW) // P
        o0 = pso.tile([P, JW], F32, name="o0", tag="o0")
        o1 = pso.tile([P, JW], F32, name="o1", tag="o1")
        ops = (o0, o1)
        pend = None

        def pv(i, d, pt):
            for k in range(2):
                nc.tensor.matmul(
                    ops[k][:, d:JW],
                    lhsT=vA[:, i * VAW:(i + 1) * VAW],
                    rhs=pt[:, k, d:JW],
                    start=(i == 0), stop=(i == ni - 1))

        for i in range(ni):
            d = max(0, i * P - n0)
            sp = pssp.tile([P, 2, JW], F32, name="sp", tag="sp")
            nc.tensor.matmul(sp[:, 0, d:JW],
                             lhsT=kr[0:HD, i * P:(i + 1) * P],
                             rhs=qrt[0:HD, n0 + d:n0 + JW],
                             start=True, stop=True)
            nc.tensor.matmul(sp[:, 1, d:JW],
                             lhsT=kr[HD:P, i * P:(i + 1) * P],
                             rhs=qrt[HD:P, n0 + d:n0 + JW],
                             start=True, stop=True)
            pt = ptp.tile([P, 2, JW], BF16, name="pt")
            nc.scalar.activation(out=pt[:, :, d:JW], in_=sp[:, :, d:JW],
                                 func=mybir.ActivationFunctionType.Exp)
            if i * P >= n0:
                nc.vector.tensor_mul(pt[:, :, d:d + P], pt[:, :, d:d + P],
                                     mask2_sb)
            if pend is not None:
                pv(*pend)
            pend = (i, d, pt)
        pv(*pend)

        def norm(n0=n0, ops=ops):
            for k in range(2):
                o_ps = ops[k]
                l_sb = nrm.tile([1, JW], F32, name="l_sb", tag="l")
                nc.vector.tensor_copy(out=l_sb, in_=o_ps[HD:HD + 1, :])
                r = nrm.tile([1, JW], F32, name="r", tag="r")
                nc.vector.reciprocal_approx_fast(out=r, in_=l_sb)
                rb16 = nrm.tile([1, JW], BF16, name="rb16", tag="r16")
                nc.vector.tensor_copy(out=rb16, in_=r)
                nc.tensor.matmul(o_ps[HD:HD + HD, :], lhsT=ones_sb,
                                 rhs=rb16, start=True, stop=True)
                o_f = otp.tile([HD, JW], F32, name="o_f", tag="o_f")
                nc.vector.tensor_copy(out=o_f, in_=o_ps[0:HD, :])
                ot = otp.tile([HD, JW], BF16, name="ot")
                nc.vector.tensor_mul(ot, o_f, o_ps[HD:HD + HD, :])
                for half in range(JW // RSH):
                    dest = (n0 + half * RSH) // RSH
                    nc.sync.dma_start(
                        out=a2a_in[b, hp, dest, k],
                        in_=ot[:, half * RSH:(half + 1) * RSH])

        if defer is not None:
            defer.append(norm)
        else:
            norm()


def _load_wos(nc, woT, pool, db, eng):
    """Stage one wo column block (single 2MB DMA); returns the tile."""
    w = pool.tile([P, CH * JW], BF16, name=f"wos{db}", tag="w")
    eng.dma_start(out=w, in_=woT[db])
    return w


def _outproj_wave(nc, db, b, hp, out, orT, wos, ops, outsp, drain=None):
    """One head-pair wave of a (column-block, batch) o @ wo.T pass."""
    MT = RSH // P
    for si, s in enumerate(range(NCORES)):
        c = 2 * s + hp
        for mt in range(MT):
            nc.tensor.matmul(
                ops[mt][:, :],
                lhsT=orT[(b, hp)][:, s * RSH + mt * P:s * RSH + (mt + 1) * P],
                rhs=wos[:, c * JW:(c + 1) * JW],
                start=(hp == 0 and si == 0),
                stop=(hp == 1 and si == NCORES - 1))
    if hp == 1:
        for mt in range(MT):
            osb = outsp.tile([P, JW], F32, name="osb")
            (drain or nc.scalar.copy)(out=osb, in_=ops[mt])
            nc.sync.dma_start(
                out=out[b * RSH + mt * P:b * RSH + (mt + 1) * P,
                        db * JW:(db + 1) * JW],
                in_=osb)


def _outproj_pass(nc, db, b, out, orT, wos, pool, outsp, drain):
    ops = [pool.tile([P, JW], F32, name=f"op{mt}", tag="ps")
           for mt in range(RSH // P)]
    for hp in range(2):
        _outproj_wave(nc, db, b, hp, out, orT, wos, ops, outsp, drain=drain)


def _host_prep(x, freqs_cis, wq, wk, wv, wo):
    """Build per-core input maps (numpy only)."""
    x = np.asarray(x, np.float32)
    freqs_cis = np.asarray(freqs_cis, np.float32)
    wq = np.asarray(wq, np.float32)
    wk = np.asarray(wk, np.float32)
    wv = np.asarray(wv, np.float32)
    wo = np.asarray(wo, np.float32)
    bf = ml_dtypes.bfloat16

    # [nbg, p, c, n]: one contiguous 16KB run per partition per n-block
    xT = np.ascontiguousarray(
        x.reshape(BS, D).T.reshape(CH, P, B * NBB, NBW)
        .transpose(2, 1, 0, 3)).astype(bf).reshape(B * NBB, P, CH * NBW)
    # [db, p, c, jw]: one DMA per wo column block
    woT = np.ascontiguousarray(
        wo.T.reshape(CH, P, D // JW, JW).transpose(2, 1, 0, 3)
        ).astype(bf).reshape(D // JW, P, CH * JW)
    scale = 1.0 / np.sqrt(np.float32(HD))

    cos = freqs_cis[:, :, 0]
    sin = freqs_cis[:, :, 1]
    pair = (np.arange(P) // 2) % (HD // 2)
    sign = np.where(np.arange(P) % 2 == 0, -1.0, 1.0).astype(np.float32)
    cosT = np.ascontiguousarray(cos[:, pair].T).astype(bf)
    sinPM = (np.ascontiguousarray(sin[:, pair].T) * sign[:, None]).astype(bf)

    maskb = (np.arange(P)[None, :] >= np.arange(P)[:, None]).astype(bf)
    mask2 = np.concatenate([maskb, maskb], axis=1)

    ident = np.eye(P, dtype=bf)

    in_maps = []
    for r in range(NCORES):
        wq_r = wq[r * EQ:(r + 1) * EQ] * scale
        wk_r = wk[r * EK:(r + 1) * EK]
        wv_r = wv[r * EV:(r + 1) * EV]
        wTn = np.ascontiguousarray(
            np.concatenate([wq_r.T, wk_r.T, wv_r.T], axis=1)
            .reshape(CH, P, E3).transpose(1, 0, 2)).astype(bf).reshape(P, CH * E3)
        in_maps.append({
            "xT": xT, "wT": wTn, "woT": woT,
            "cosT": cosT, "sinPM": sinPM, "mask2": mask2, "ident": ident,
        })
    return in_maps


def kernel(x, freqs_cis, wq, wk, wv, wo):
    if "nc" not in _CACHE:
        _CACHE["nc"] = _build_nc()
    nc = _CACHE["nc"]

    in_maps = _host_prep(x, freqs_cis, wq, wk, wv, wo)
    trace = bool(int(os.environ.get("KPROF", "0")))
    res = run_bass_kernel_spmd(nc, in_maps, core_ids=list(range(NCORES)),
                               trace=trace)
    if trace:
        _CACHE["last_results"] = res

    full = np.empty((BS, D), np.float32)
    for r in range(NCORES):
        o = res.results[r]["out"]
        full[r * RSH:(r + 1) * RSH] = o[0:RSH]
        full[S + r * RSH:S + (r + 1) * RSH] = o[RSH:2 * RSH]
    return full.reshape(B, S, D)


if __name__ == "__main__":
    rng = np.random.default_rng(0)
    ins = {
        "x": rng.standard_normal((B, S, D), np.float32),
        "freqs_cis": rng.standard_normal((S, HD // 2, 2), np.float32),
        "wq": (rng.standard_normal((H * HD, D)) * 0.02).astype(np.float32),
        "wk": (rng.standard_normal((KV * HD, D)) * 0.02).astype(np.float32),
        "wv": (rng.standard_normal((KV * HD, D)) * 0.02).astype(np.float32),
        "wo": (rng.standard_normal((D, H * HD)) * 0.02).astype(np.float32),
    }
    out = kernel(**ins)
    print("kernel ran, out shape", out.shape, "finite:", np.isfinite(out).all())
